# revision 50
# baseline (speedup 1.0000x reference)
"""GAT-style sparse neighbor aggregation kernel for Trainium2 (8 NeuronCores).

Reference computation (dense):
    hf = X @ W; he = E @ W
    e  = leakyrelu((hf@a1)[:,None] + (he@a2)[None,:])
    att = softmax(where(mask, e, -9e15), axis=1)     # mask: <=10 nnz/row
    out = att @ he

Key algebra: att @ he == (att @ E) @ W, and att is row-sparse (<=K nnz).
So per row i:  out_i = (sum_k w_ik * E[idx_ik]) @ W   with
    s_ik = leakyrelu(f_i + g_ik),  f = X @ (W@a1),  g_ik = E[idx_ik]. (W@a2)
    w_ik = softmax over the deduplicated k's.
This turns 56 GFLOP dense into ~5 GFLOP total.

The dominant cost at this scale is HOST<->DEVICE wire traffic (the trn2
cores are axon-tunneled, ~45-60MB/s), so the kernel minimizes bytes:
  - batch rows N=2048 split across 8 cores (256 rows each)
  - embed table E and weight W are NOT replicated: each core uploads a
    1/8 row-shard and the full tensors are rebuilt on-device with an
    AllGather collective over NeuronLink (~300MB/call -> ~12MB/call).
  - E crosses the wire as int8 with per-row scales; the scales are
    folded into the attention weights on device (agg = sum_k (w_k*s_k)
    * Ê_k), so no dequantized copy of E is ever materialized. W is
    fp16. The output returns as int8 + per-row scale and is dequantized
    on host. Measured rel err ~9.8e-3 vs the 2e-2 gate.
  - f = X @ (W@a1) (the per-row logit offset, 8KB) is precomputed on
    host instead of uploading the 8MB feature matrix.
  - transfer shaping: the emb shard uploads as 4 chunks (pipelines
    quantize->wire and raises tunnel stream concurrency ~20%), and all
    per-row metadata (fv, dup mask, dequant scales, idx as int32 bits)
    rides in ONE packed [NL, 32] f32 array — each extra sharded array
    costs ~10ms of tunnel overhead.

Engine mapping per core:
  - AllGather(w fp16), AllGather(emb int8) over internal DRAM tiles
  - gathers: gpsimd indirect DMA of int8 rows from the gathered table
  - dot products (a2'=W@a2, g=Ê.a2'): DVE tensor_mul + ACT accum-reduce
    (fp32, DVE casts from int8/fp16)
  - softmax over k: DVE small ops + ACT fused exp/sum
  - aggregation sum_k (w*s)*Ê AND its transpose: PE fp16 matmuls
    (lhsT=Ê chunk, rhs=diag(w*s)) accumulating aggT directly in PSUM
  - final agg @ W: PE fp16 matmuls (N=512 free dim), then per-row
    abs-max + scale to int8 on DVE
"""

import os
import sys

import numpy as np

sys.path.insert(0, "/opt/trn_rl_repo")

from contextlib import ExitStack

import concourse.bass as bass
import concourse.tile as tile
from concourse import bacc, mybir
from concourse.bass_utils import run_bass_kernel_spmd
from concourse.masks import make_identity

N, M, F, K = 2048, 8192, 1024, 10
NCORES = 8
NL = N // NCORES  # 256 rows per core
P = 128
T = NL // P  # row-tiles per core (2)
FC = F // P  # feature chunks (8)
MSH = M // NCORES  # emb rows per core shard (1024)
WSH = F // NCORES  # weight rows per core shard (128)
NH = 512  # matmul free-dim half (psum bank)
EMB_CH = 4  # upload chunks per emb shard (tunnel parallelism)
CHR = MSH // EMB_CH  # rows per upload chunk per core (256)
ALPHA = 0.2
NEGBIG = -1e30

f32 = mybir.dt.float32
f16 = mybir.dt.float16
i32 = mybir.dt.int32
i8 = mybir.dt.int8
AX = mybir.AxisListType
OP = mybir.AluOpType
ACT = mybir.ActivationFunctionType

RG = [list(range(NCORES))]


def build_kernel():
    nc = bacc.Bacc("TRN2", target_bir_lowering=False, debug=False, num_devices=NCORES)

    w_sh = nc.dram_tensor("w_sh", [WSH, F], f16, kind="ExternalInput").ap()
    # emb shard arrives in EMB_CH chunks so the host can pipeline
    # quantize->upload and the tunnel runs more concurrent streams
    emb_shc = [
        nc.dram_tensor(f"emb_sh{c}", [CHR, F], i8, kind="ExternalInput").ap()
        for c in range(EMB_CH)
    ]
    # packed per-row metadata, one upload instead of five:
    #   col 0: fv (f_i = X@(W@a1)); 1..11: dup-mask add; 11..21: emb dequant
    #   scales s_{idx[i,k]}; 21..31: idx as int32 bits; 31: pad;
    #   32..36: a2' = W@a2 as row-blocks (a2p[4r:4r+4] in row r, replicated)
    pk = nc.dram_tensor("pk", [NL, 36], f32, kind="ExternalInput").ap()
    out8 = nc.dram_tensor("out8", [NL, F], i8, kind="ExternalOutput").ap()
    osc = nc.dram_tensor("osc", [NL, 1], f32, kind="ExternalOutput").ap()

    with tile.TileContext(nc) as tc, ExitStack() as ctx:
        const = ctx.enter_context(tc.tile_pool(name="const", bufs=1))
        big = ctx.enter_context(tc.tile_pool(name="big", bufs=1))
        eg_pool = ctx.enter_context(tc.tile_pool(name="eg", bufs=2))
        sm = ctx.enter_context(tc.tile_pool(name="small", bufs=2))
        scr = ctx.enter_context(tc.tile_pool(name="scratch", bufs=4))
        dg = ctx.enter_context(tc.tile_pool(name="diag", bufs=2 * K + 2))
        ps = ctx.enter_context(tc.tile_pool(name="psum", bufs=3, space="PSUM"))
        pso = ctx.enter_context(tc.tile_pool(name="psum_o", bufs=2, space="PSUM"))
        dram = ctx.enter_context(tc.tile_pool(name="dram", bufs=1, space="DRAM"))

        # ---- rebuild full W and E on-device from 1/8 shards (AllGather) ----
        w_in = dram.tile([WSH, F], f16)
        w_full = dram.tile([F, F], f16, addr_space="Shared")
        nc.sync.dma_start(w_in[:], w_sh)
        nc.gpsimd.collective_compute(
            "AllGather", OP.bypass, replica_groups=RG,
            ins=[w_in[:].opt()], outs=[w_full[:].opt()],
        )
        emb_in = dram.tile([MSH, F], i8)
        emb_full = dram.tile([M, F], i8, addr_space="Shared")
        for c in range(EMB_CH):
            nc.sync.dma_start(emb_in[c * CHR : (c + 1) * CHR, :], emb_shc[c])
        nc.gpsimd.collective_compute(
            "AllGather", OP.bypass, replica_groups=RG,
            ins=[emb_in[:].opt()], outs=[emb_full[:].opt()],
        )

        nonce = os.environ.get("KERNEL_NONCE")
        if nonce:  # testing only: force a distinct NEFF hash per nonce
            nt = const.tile([P, 8], f32, name=f"nonce_{nonce}")
            nc.gpsimd.memset(nt[:], float(len(nonce)))

        ident = const.tile([P, P], f16)
        make_identity(nc, ident[:])

        # W resident in SBUF as fp16: w_sb[p, c, j] = W[c*128+p, j]
        w_sb = big.tile([P, FC, F], f16)
        nc.sync.dma_start(w_sb[:], w_full[:].rearrange("(c p) j -> p c j", p=P))

        # packed per-row metadata block (see pk layout above)
        pk_sb = big.tile([P, T, 36], f32)
        nc.sync.dma_start(pk_sb[:], pk.rearrange("(t p) c -> p t c", p=P))
        pk_i32 = pk_sb[:].bitcast(i32)

        def dot(in0, in1, acc_slice):
            """acc_slice[p, 0] = sum_j in0[p, j] * in1[p, j] (DVE mult + ACT reduce)."""
            m = scr.tile([P, F], f32, tag="mul")
            nc.vector.tensor_mul(out=m[:], in0=in0, in1=in1)
            dmy = sm.tile([P, 1], f32, tag="dummy")
            nc.scalar.activation(
                out=dmy[:].broadcast_to(m[:].shape), in_=m[:],
                func=ACT.Identity, bias=0.0, scale=1.0, accum_out=acc_slice,
            )

        # a2' = W @ a2 precomputed on host (exact fp32), broadcast from the
        # packed block's 32..36 columns
        a2pb = big.tile([P, F], f32)
        nc.sync.dma_start(
            a2pb[:].rearrange("p (r c) -> p r c", c=4),
            pk[:, 32:36].unsqueeze(0).partition_broadcast(P),
        )

        aggT = big.tile([P, T, FC, P], f16)

        for t in range(T):
            r0 = t * P
            dn_t = pk_sb[:, t, 1 : 1 + K]
            esc_t = pk_sb[:, t, 11 : 11 + K]

            # gather int8 embed rows: eg8[p, k, :] = E8[idx[r0+p, k], :]
            eg8 = eg_pool.tile([P, K, F], i8, tag="eg8")
            for k in range(K):
                nc.gpsimd.indirect_dma_start(
                    out=eg8[:, k, :],
                    out_offset=None,
                    in_=emb_full[:],
                    in_offset=bass.IndirectOffsetOnAxis(
                        ap=pk_i32[:, t, 21 + k : 22 + k], axis=0
                    ),
                )
            # int8 -> fp16 for the PE (scales are folded into the diag weights)
            eg = eg_pool.tile([P, K, F], f16, tag="eg")
            nc.vector.tensor_copy(out=eg[:], in_=eg8[:])

            # g_ik = s_ik * (Ê[i,k,:] . a2') ; f_i = X[i,:] . (W@a1)
            g_t = sm.tile([P, K], f32, tag="g")
            for k in range(K):
                ekf = scr.tile([P, F], f32, tag="ekf")
                nc.vector.tensor_copy(out=ekf[:], in_=eg8[:, k, :])
                dot(ekf[:], a2pb[:], g_t[:, k : k + 1])
            nc.vector.tensor_tensor(out=g_t[:], in0=g_t[:], in1=esc_t, op=OP.mult)

            # scores: s = leakyrelu(g + f) + dup_mask_neg
            s_t = sm.tile([P, K], f32, tag="s")
            nc.vector.tensor_scalar_add(out=s_t[:], in0=g_t[:], scalar1=pk_sb[:, t, 0:1])
            lr = sm.tile([P, K], f32, tag="lr")
            nc.vector.tensor_scalar_mul(out=lr[:], in0=s_t[:], scalar1=ALPHA)
            nc.vector.tensor_tensor(out=s_t[:], in0=s_t[:], in1=lr[:], op=OP.max)
            nc.vector.tensor_tensor(out=s_t[:], in0=s_t[:], in1=dn_t, op=OP.add)

            # masked softmax over k (exp and normalizer fused on ACT)
            mx = sm.tile([P, 1], f32, tag="mx")
            nc.vector.tensor_reduce(out=mx[:], in_=s_t[:], axis=AX.X, op=OP.max)
            nmx = sm.tile([P, 1], f32, tag="nmx")
            nc.vector.tensor_scalar_mul(out=nmx[:], in0=mx[:], scalar1=-1.0)
            p_t = sm.tile([P, K], f32, tag="p")
            z_t = sm.tile([P, 1], f32, tag="z")
            nc.scalar.activation(
                out=p_t[:], in_=s_t[:], func=ACT.Exp, bias=nmx[:], scale=1.0,
                accum_out=z_t[:],
            )
            zi = sm.tile([P, 1], f32, tag="zi")
            nc.vector.reciprocal(out=zi[:], in_=z_t[:])
            wts = sm.tile([P, K], f32, tag="wts")
            nc.vector.tensor_scalar_mul(out=wts[:], in0=p_t[:], scalar1=zi[:])
            # fold dequant scales into the aggregation weights:
            #   agg_i = sum_k (w_ik * s_ik) * Ê[idx_ik]
            nc.vector.tensor_tensor(out=wts[:], in0=wts[:], in1=esc_t, op=OP.mult)

            # diag(w) tiles (fp16 for the PE)
            dks = []
            for k in range(K):
                dk = dg.tile([P, P], f16, tag="dk")
                nc.vector.tensor_scalar_mul(out=dk[:], in0=ident[:], scalar1=wts[:, k : k + 1])
                dks.append(dk)

            # aggregation, transposed directly:
            #   aggT[m, n] = sum_k (eg[:, k, c*128+m]).T @ diag(w_k) = w_n * E[idx[n,k], c*128+m]
            for c in range(FC):
                at_ps = ps.tile([P, P], f32, tag="at_ps")
                for k in range(K):
                    nc.tensor.matmul(
                        out=at_ps[:],
                        lhsT=eg[:, k, c * P : (c + 1) * P],
                        rhs=dks[k][:],
                        start=(k == 0),
                        stop=(k == K - 1),
                    )
                nc.vector.tensor_copy(out=aggT[:, t, c, :], in_=at_ps[:])

            # out = agg @ W: out[r, j] = sum_c aggT[:, t, c, r] . W-chunk
            obf = scr.tile([P, F], f32, tag="obf")
            for nh in range(F // NH):
                o_ps = pso.tile([P, NH], f32, tag="o_ps")
                for c in range(FC):
                    nc.tensor.matmul(
                        out=o_ps[:],
                        lhsT=aggT[:, t, c, :],
                        rhs=w_sb[:, c, nh * NH : (nh + 1) * NH],
                        start=(c == 0),
                        stop=(c == FC - 1),
                    )
                nc.vector.tensor_copy(out=obf[:, nh * NH : (nh + 1) * NH], in_=o_ps[:])

            # int8 per-row quantization of the output (halves the slow D2H):
            #   osc_i = max|out_i|/127,  out8_i = out_i / osc_i
            omx = sm.tile([P, 1], f32, tag="omx")
            nc.vector.tensor_reduce(out=omx[:], in_=obf[:], axis=AX.X, op=OP.max)
            omn = sm.tile([P, 1], f32, tag="omn")
            nc.vector.tensor_reduce(out=omn[:], in_=obf[:], axis=AX.X, op=OP.min)
            nmn = sm.tile([P, 1], f32, tag="nmn")
            nc.vector.tensor_scalar_mul(out=nmn[:], in0=omn[:], scalar1=-1.0)
            rmax = sm.tile([P, 1], f32, tag="rmax")
            nc.vector.tensor_tensor(out=rmax[:], in0=omx[:], in1=nmn[:], op=OP.max)
            osc_t = sm.tile([P, 1], f32, tag="osc")
            nc.vector.tensor_scalar_mul(out=osc_t[:], in0=rmax[:], scalar1=1.0 / 127.0)
            nc.vector.tensor_scalar_max(out=osc_t[:], in0=osc_t[:], scalar1=1e-12)
            oinv = sm.tile([P, 1], f32, tag="oinv")
            nc.vector.reciprocal(out=oinv[:], in_=osc_t[:])
            o8 = scr.tile([P, F], i8, tag="o8")
            nc.vector.tensor_scalar_mul(out=o8[:], in0=obf[:], scalar1=oinv[:])
            nc.sync.dma_start(out8[r0 : r0 + P, :], o8[:])
            nc.sync.dma_start(osc[r0 : r0 + P, :], osc_t[:])

    nc.compile()
    return nc


_NC_CACHE = None


def _get_nc():
    global _NC_CACHE
    if _NC_CACHE is None:
        _NC_CACHE = build_kernel()
    return _NC_CACHE


def _host_prep(feature_matrix, embed_matrix, weight, a, neigh_idx, put=None):
    """Global (already concat-ordered) per-name arrays for the sharded runner.

    Every tensor is sharded on axis 0 across the 8 cores, so the full input
    arrays ARE the concatenation of per-core shards — no host copies beyond
    the fp16 casts. Only `av2` (replicated) needs tiling.

    If `put` is given, each big array is handed to it as soon as it is ready
    (async jax.device_put) so the wire transfer overlaps the remaining host
    prep.
    """
    if put is None:
        put = lambda x: x
    out = {}
    emb32 = np.asarray(embed_matrix, dtype=np.float32)
    # int8 per-row symmetric quantization of the embed table. Chunked so each
    # upload chunk starts its (async) wire transfer as soon as it is
    # quantized, and the tunnel runs several concurrent streams (~20% faster
    # than one large transfer). The per-row scales travel separately and are
    # folded into the attention weights on device, so only 1 byte/element
    # crosses the wire.
    esc = np.empty((M, 1), np.float32)
    import concurrent.futures as _cf

    def _quant_part(args):
        k, c, dst = args
        sl = slice(k * MSH + c * CHR, k * MSH + (c + 1) * CHR)
        ch = emb32[sl]
        sc = np.maximum(np.abs(ch).max(axis=1, keepdims=True), 1e-12) / 127.0
        dst[k * CHR : (k + 1) * CHR] = np.clip(np.round(ch / sc), -127, 127)
        esc[sl] = sc

    with _cf.ThreadPoolExecutor(8) as ex:
        for c in range(EMB_CH):
            dst = np.empty((NCORES * CHR, F), np.int8)
            list(ex.map(_quant_part, [(k, c, dst) for k in range(NCORES)]))
            out[f"emb_sh{c}"] = put(dst)
    w32 = np.asarray(weight, dtype=np.float32)
    out["w_sh"] = put(np.ascontiguousarray(w32.astype(np.float16)))

    feat32 = np.asarray(feature_matrix, dtype=np.float32)
    av = np.asarray(a, dtype=np.float32).reshape(2 * F)

    idx = np.asarray(neigh_idx)
    idx32 = idx.astype(np.int32)
    # duplicate-index mask (set semantics): only first occurrence is valid
    dup = np.zeros((N, K), dtype=bool)
    for k in range(1, K):
        dup[:, k] = (idx[:, :k] == idx[:, k : k + 1]).any(axis=1)

    # packed per-row metadata (one upload instead of four — each extra
    # sharded array costs ~10ms of tunnel overhead):
    pk = np.zeros((N, 36), np.float32)
    # f_i = X[i] . (W @ a1) — per-row logit offset (8KB instead of a 4MB
    # feature upload; the a2 half of the attention vector stays on device)
    pk[:, 0] = feat32 @ (w32 @ av[:F])
    pk[:, 1 : 1 + K] = np.where(dup, np.float32(NEGBIG), np.float32(0.0))
    pk[:, 11 : 11 + K] = esc[idx32, 0]  # per-(row,k) dequant scales
    pk[:, 21 : 21 + K] = idx32.view(np.float32)  # int32 bits, bitcast on device
    pk[:, 32:36] = np.tile((w32 @ av[F:]).astype(np.float32).reshape(NL, 4), (NCORES, 1))
    out["pk"] = put(pk)
    return out


_RUNNER_CACHE = None


def _make_runner(nc):
    """jit(shard_map(bass_exec)) caller, like bass2jax.run_bass_via_pjrt but:
    - zero output buffers are created ON DEVICE (no host->device zeros upload)
    - takes global arrays directly (no per-core split + re-concat on host)
    """
    import jax
    import jax.numpy as jnp
    from jax.experimental.shard_map import shard_map
    from jax.sharding import Mesh, PartitionSpec

    from concourse import bass2jax, mybir as _mybir
    from concourse.bass2jax import (
        _bass_exec_p,
        install_neuronx_cc_hook,
        partition_id_tensor,
    )

    install_neuronx_cc_hook()

    partition_name = nc.partition_id_tensor.name if nc.partition_id_tensor else None
    in_names, out_names, out_avals = [], [], []
    for alloc in nc.m.functions[0].allocations:
        if not isinstance(alloc, _mybir.MemoryLocationSet):
            continue
        name = alloc.memorylocations[0].name
        if alloc.kind == "ExternalInput":
            if name != partition_name:
                in_names.append(name)
        elif alloc.kind == "ExternalOutput":
            shape = tuple(alloc.tensor_shape)
            dtype = _mybir.dt.np(alloc.dtype)
            out_names.append(name)
            out_avals.append(jax.core.ShapedArray(shape, dtype))
    n_params = len(in_names)
    all_names = list(in_names) + list(out_names)
    if partition_name is not None:
        all_names.append(partition_name)

    def _body(*args):
        operands = list(args)
        if partition_name is not None:
            operands.append(partition_id_tensor())
        outs = _bass_exec_p.bind(
            *operands,
            out_avals=tuple(out_avals),
            in_names=tuple(all_names),
            out_names=tuple(out_names),
            lowering_input_output_aliases=(),
            sim_require_finite=True,
            sim_require_nnan=True,
            nc=nc,
        )
        return tuple(outs)

    devices = jax.devices()[:NCORES]
    mesh = Mesh(np.asarray(devices), ("core",))
    n_outs = len(out_names)
    sharded = jax.jit(
        shard_map(
            _body,
            mesh=mesh,
            in_specs=(PartitionSpec("core"),) * (n_params + n_outs),
            out_specs=(PartitionSpec("core"),) * n_outs,
            check_rep=False,
        )
    )

    # Persistent device-resident zero buffers for the NEFF output slots:
    # uploaded once here, reused every call (NOT donated, so never
    # invalidated). The kernel writes every element of every output, so
    # stale contents between calls are harmless.
    from jax.sharding import NamedSharding

    sh = NamedSharding(mesh, PartitionSpec("core"))
    zeros_dev = [
        jax.device_put(
            np.zeros((NCORES * a.shape[0], *a.shape[1:]), a.dtype), sh
        )
        for a in out_avals
    ]

    def call(name_map):
        args = [name_map[n] for n in in_names]
        outs = sharded(*args, *zeros_dev)
        return {n: outs[i] for i, n in enumerate(out_names)}

    return call, sh


def _get_runner():
    global _RUNNER_CACHE
    if _RUNNER_CACHE is None:
        _RUNNER_CACHE = _make_runner(_get_nc())
    return _RUNNER_CACHE


def run(inputs, trace=False, **kw):
    if trace:  # profiling path: go through the stock spmd runner
        nc = _get_nc()
        g = _host_prep(**inputs)
        in_maps = []
        for c in range(NCORES):
            m = {
                f"emb_sh{cc}": g[f"emb_sh{cc}"][c * CHR : (c + 1) * CHR]
                for cc in range(EMB_CH)
            }
            in_maps.append(
                {
                    **m,
                    "w_sh": g["w_sh"][c * WSH : (c + 1) * WSH],
                    "pk": g["pk"][c * NL : (c + 1) * NL],
                }
            )
        res = run_bass_kernel_spmd(
            nc, in_maps, core_ids=list(range(NCORES)), trace=trace, **kw
        )
        out = np.concatenate(
            [
                res.results[c]["out8"].astype(np.float32) * res.results[c]["osc"]
                for c in range(NCORES)
            ],
            axis=0,
        )
        return out, res
    global _DEVICE_POISONED
    if not _DEVICE_POISONED:
        try:
            return _run_device(inputs), None
        except Exception as e:
            # A device-unrecoverable error poisons this process's PJRT client
            # for good; all further device work goes through a fresh-process
            # worker (which CAN recover, since it gets a fresh client).
            sys.stderr.write(
                f"kernel: in-process device call failed ({type(e).__name__}); "
                f"switching to subprocess worker\n"
            )
            _DEVICE_POISONED = True
    prepped = _host_prep(**inputs)
    for attempt in range(2):
        try:
            return _worker_call(prepped), None
        except Exception as e:
            sys.stderr.write(
                f"kernel: worker attempt {attempt} failed ({type(e).__name__})\n"
            )
            _kill_worker()
    sys.stderr.write("kernel: all device paths failed; host fallback\n")
    return _numpy_fallback(**inputs), None


def _run_device(inputs):
    import concurrent.futures as _cf

    import jax

    call, sh = _get_runner()
    put = lambda arr: jax.device_put(arr, sh)
    outs = call(_host_prep(**inputs, put=put))
    return _fetch_out(outs)


def _fetch_out(outs):
    import concurrent.futures as _cf

    # start all shard->host copies, then fetch both outputs concurrently
    # (each D2H fetch has ~0.1s fixed latency)
    for s in outs["out8"].addressable_shards:
        try:
            s.data.copy_to_host_async()
        except Exception:
            break
    with _cf.ThreadPoolExecutor(2) as ex:
        f8 = ex.submit(np.asarray, outs["out8"])
        fs = ex.submit(np.asarray, outs["osc"])
        o8, sc = f8.result(), fs.result()
    return o8.astype(np.float32) * sc


def _exec_prepped(prepped):
    """Device execution for an already-prepped numpy dict (worker path)."""
    import jax

    call, sh = _get_runner()
    staged = {k: jax.device_put(v, sh) for k, v in prepped.items()}
    return _fetch_out(call(staged))


_DEVICE_POISONED = False
_WORKER = None


def _spawn_worker():
    import subprocess

    here = os.path.dirname(os.path.abspath(__file__))
    boot = (
        "import sys; "
        f"sys.path.insert(0, {here!r}); "
        "sys.path.insert(0, '/opt/trn_rl_repo'); "
        "import kernel; kernel._worker_main()"
    )
    return subprocess.Popen(
        [sys.executable, "-u", "-c", boot],
        stdin=subprocess.PIPE,
        stdout=subprocess.PIPE,
        stderr=sys.stderr,
        text=True,
    )


def _kill_worker():
    global _WORKER
    if _WORKER is not None:
        try:
            _WORKER.kill()
        except Exception:
            pass
        _WORKER = None


def _worker_call(prepped, timeout=420.0):
    global _WORKER
    import tempfile
    import threading

    if _WORKER is None or _WORKER.poll() is not None:
        _kill_worker()
        _WORKER = _spawn_worker()
        import atexit

        atexit.register(_kill_worker)
    proc = _WORKER
    tmp = tempfile.mkdtemp()
    in_path = os.path.join(tmp, "in.npz")
    out_path = os.path.join(tmp, "out.npy")
    np.savez(in_path, **prepped)
    proc.stdin.write(f"CALL {in_path} {out_path}\n")
    proc.stdin.flush()
    timer = threading.Timer(timeout, proc.kill)
    timer.start()
    try:
        while True:
            line = proc.stdout.readline()
            if not line:
                raise RuntimeError("worker died")
            if line.startswith("DONE"):
                return np.load(out_path)
            if line.startswith("FAIL"):
                raise RuntimeError(f"worker: {line.strip()}")
            # anything else is stray log noise on stdout; skip it
    finally:
        timer.cancel()


def _worker_main():
    """Entry point for the device-worker subprocess (fresh PJRT client)."""
    for line in sys.stdin:
        parts = line.split()
        if not parts:
            continue
        if parts[0] == "QUIT":
            break
        if parts[0] != "CALL" or len(parts) != 3:
            continue
        in_path, out_path = parts[1], parts[2]
        try:
            data = np.load(in_path)
            prepped = {k: data[k] for k in data.files}
            out = _exec_prepped(prepped)
            np.save(out_path, out)
            sys.stdout.write("DONE\n")
        except Exception as e:
            sys.stdout.write(f"FAIL {type(e).__name__}: {str(e)[:120]}".replace("\n", " ") + "\n")
        sys.stdout.flush()


def _numpy_fallback(feature_matrix, embed_matrix, weight, a, neigh_idx):
    """Exceptional-path-only host implementation (exact, sparse formulation)."""
    X = np.asarray(feature_matrix, dtype=np.float32)
    E = np.asarray(embed_matrix, dtype=np.float32)
    W = np.asarray(weight, dtype=np.float32)
    av = np.asarray(a, dtype=np.float32).reshape(2 * F)
    idx = np.asarray(neigh_idx)
    fv = X @ (W @ av[:F])
    a2p = W @ av[F:]
    eg = E[idx]  # [N, K, F]
    g = eg @ a2p
    s = fv[:, None] + g
    s = np.where(s > 0, s, np.float32(ALPHA) * s)
    dup = np.zeros((N, K), dtype=bool)
    for k in range(1, K):
        dup[:, k] = (idx[:, :k] == idx[:, k : k + 1]).any(axis=1)
    s = np.where(dup, np.float32(NEGBIG), s)
    s = s - s.max(axis=1, keepdims=True)
    p = np.exp(s)
    p /= p.sum(axis=1, keepdims=True)
    agg = np.einsum("nk,nkf->nf", p, eg)
    return (agg @ W).astype(np.float32)


def kernel(**inputs) -> np.ndarray:
    out, _ = run(inputs, trace=False)
    return out


# revision 51
# speedup vs baseline: 1.3683x; 1.3683x over previous
"""GAT-style sparse neighbor aggregation kernel for Trainium2 (8 NeuronCores).

Reference computation (dense):
    hf = X @ W; he = E @ W
    e  = leakyrelu((hf@a1)[:,None] + (he@a2)[None,:])
    att = softmax(where(mask, e, -9e15), axis=1)     # mask: <=10 nnz/row
    out = att @ he

Key algebra: att @ he == (att @ E) @ W, and att is row-sparse (<=K nnz).
So per row i:  out_i = (sum_k w_ik * E[idx_ik]) @ W   with
    s_ik = leakyrelu(f_i + g_ik),  f = X @ (W@a1),  g_ik = E[idx_ik]. (W@a2)
    w_ik = softmax over the deduplicated k's.
This turns 56 GFLOP dense into ~5 GFLOP total.

The dominant cost at this scale is HOST<->DEVICE wire traffic (the trn2
cores are axon-tunneled, ~45-60MB/s), so the kernel minimizes bytes:
  - batch rows N=2048 split across 8 cores (256 rows each)
  - embed table E and weight W are NOT replicated: each core uploads a
    1/8 row-shard and the full tensors are rebuilt on-device with an
    AllGather collective over NeuronLink (~300MB/call -> ~12MB/call).
  - E crosses the wire as int8 with per-row scales; the scales are
    folded into the attention weights on device (agg = sum_k (w_k*s_k)
    * Ê_k), so no dequantized copy of E is ever materialized. W is
    fp16. The output returns as int8 + per-row scale and is dequantized
    on host. Measured rel err ~9.8e-3 vs the 2e-2 gate.
  - f = X @ (W@a1) (the per-row logit offset, 8KB) is precomputed on
    host instead of uploading the 8MB feature matrix.
  - transfer shaping: the emb shard uploads as 4 chunks (pipelines
    quantize->wire and raises tunnel stream concurrency ~20%), and all
    per-row metadata (fv, dup mask, dequant scales, idx as int32 bits,
    and host-computed a2'=W@a2) rides in ONE packed [NL, 36] f32 array — each extra sharded array
    costs ~10ms of tunnel overhead.

Engine mapping per core:
  - AllGather(w fp16), AllGather(emb int8) over internal DRAM tiles
  - gathers: gpsimd indirect DMA of int8 rows from the gathered table
  - dot products (a2'=W@a2, g=Ê.a2'): DVE tensor_mul + ACT accum-reduce
    (fp32, DVE casts from int8/fp16)
  - softmax over k: DVE small ops + ACT fused exp/sum
  - aggregation sum_k (w*s)*Ê AND its transpose: PE fp16 matmuls
    (lhsT=Ê chunk, rhs=diag(w*s)) accumulating aggT directly in PSUM
  - final agg @ W: PE fp16 matmuls (N=512 free dim), then per-row
    abs-max + scale to int8 on DVE
"""

import os
import sys

import numpy as np

sys.path.insert(0, "/opt/trn_rl_repo")

from contextlib import ExitStack

import concourse.bass as bass
import concourse.tile as tile
from concourse import bacc, mybir
from concourse.bass_utils import run_bass_kernel_spmd
from concourse.masks import make_identity

N, M, F, K = 2048, 8192, 1024, 10
NCORES = 8
NL = N // NCORES  # 256 rows per core
P = 128
T = NL // P  # row-tiles per core (2)
FC = F // P  # feature chunks (8)
MSH = M // NCORES  # emb rows per core shard (1024)
WSH = F // NCORES  # weight rows per core shard (128)
NH = 512  # matmul free-dim half (psum bank)
EMB_CH = 4  # upload chunks per emb shard (tunnel parallelism)
CHR = MSH // EMB_CH  # rows per upload chunk per core (256)
ALPHA = 0.2
NEGBIG = -1e30

f32 = mybir.dt.float32
f16 = mybir.dt.float16
i32 = mybir.dt.int32
i8 = mybir.dt.int8
AX = mybir.AxisListType
OP = mybir.AluOpType
ACT = mybir.ActivationFunctionType

RG = [list(range(NCORES))]


def build_kernel():
    nc = bacc.Bacc("TRN2", target_bir_lowering=False, debug=False, num_devices=NCORES)

    w_sh = nc.dram_tensor("w_sh", [WSH, F], f16, kind="ExternalInput").ap()
    # emb shard arrives in EMB_CH chunks so the host can pipeline
    # quantize->upload and the tunnel runs more concurrent streams
    emb_shc = [
        nc.dram_tensor(f"emb_sh{c}", [CHR, F], i8, kind="ExternalInput").ap()
        for c in range(EMB_CH)
    ]
    # packed per-row metadata, one upload instead of five:
    #   col 0: fv (f_i = X@(W@a1)); 1..11: dup-mask add; 11..21: emb dequant
    #   scales s_{idx[i,k]}; 21..31: idx as int32 bits; 31: pad;
    #   32..36: a2' = W@a2 as row-blocks (a2p[4r:4r+4] in row r, replicated)
    pk = nc.dram_tensor("pk", [NL, 36], f32, kind="ExternalInput").ap()
    out8 = nc.dram_tensor("out8", [NL, F], i8, kind="ExternalOutput").ap()
    osc = nc.dram_tensor("osc", [NL, 1], f32, kind="ExternalOutput").ap()

    with tile.TileContext(nc) as tc, ExitStack() as ctx:
        const = ctx.enter_context(tc.tile_pool(name="const", bufs=1))
        big = ctx.enter_context(tc.tile_pool(name="big", bufs=1))
        eg_pool = ctx.enter_context(tc.tile_pool(name="eg", bufs=2))
        sm = ctx.enter_context(tc.tile_pool(name="small", bufs=2))
        scr = ctx.enter_context(tc.tile_pool(name="scratch", bufs=4))
        dg = ctx.enter_context(tc.tile_pool(name="diag", bufs=2 * K + 2))
        ps = ctx.enter_context(tc.tile_pool(name="psum", bufs=3, space="PSUM"))
        pso = ctx.enter_context(tc.tile_pool(name="psum_o", bufs=2, space="PSUM"))
        dram = ctx.enter_context(tc.tile_pool(name="dram", bufs=1, space="DRAM"))

        # ---- rebuild full W and E on-device from 1/8 shards (AllGather) ----
        w_in = dram.tile([WSH, F], f16)
        w_full = dram.tile([F, F], f16, addr_space="Shared")
        nc.sync.dma_start(w_in[:], w_sh)
        nc.gpsimd.collective_compute(
            "AllGather", OP.bypass, replica_groups=RG,
            ins=[w_in[:].opt()], outs=[w_full[:].opt()],
        )
        emb_in = dram.tile([MSH, F], i8)
        emb_full = dram.tile([M, F], i8, addr_space="Shared")
        for c in range(EMB_CH):
            nc.sync.dma_start(emb_in[c * CHR : (c + 1) * CHR, :], emb_shc[c])
        nc.gpsimd.collective_compute(
            "AllGather", OP.bypass, replica_groups=RG,
            ins=[emb_in[:].opt()], outs=[emb_full[:].opt()],
        )

        nonce = os.environ.get("KERNEL_NONCE")
        if nonce:  # testing only: force a distinct NEFF hash per nonce
            nt = const.tile([P, 8], f32, name=f"nonce_{nonce}")
            nc.gpsimd.memset(nt[:], float(len(nonce)))

        ident = const.tile([P, P], f16)
        make_identity(nc, ident[:])

        # W resident in SBUF as fp16: w_sb[p, c, j] = W[c*128+p, j]
        w_sb = big.tile([P, FC, F], f16)
        nc.sync.dma_start(w_sb[:], w_full[:].rearrange("(c p) j -> p c j", p=P))

        # packed per-row metadata block (see pk layout above)
        pk_sb = big.tile([P, T, 36], f32)
        nc.sync.dma_start(pk_sb[:], pk.rearrange("(t p) c -> p t c", p=P))
        pk_i32 = pk_sb[:].bitcast(i32)

        def dot(in0, in1, acc_slice):
            """acc_slice[p, 0] = sum_j in0[p, j] * in1[p, j] (DVE mult + ACT reduce)."""
            m = scr.tile([P, F], f32, tag="mul")
            nc.vector.tensor_mul(out=m[:], in0=in0, in1=in1)
            dmy = sm.tile([P, 1], f32, tag="dummy")
            nc.scalar.activation(
                out=dmy[:].broadcast_to(m[:].shape), in_=m[:],
                func=ACT.Identity, bias=0.0, scale=1.0, accum_out=acc_slice,
            )

        # a2' = W @ a2 precomputed on host (exact fp32), broadcast from the
        # packed block's 32..36 columns
        a2pb = big.tile([P, F], f32)
        nc.sync.dma_start(
            a2pb[:].rearrange("p (r c) -> p r c", c=4),
            pk[:, 32:36].unsqueeze(0).partition_broadcast(P),
        )

        aggT = big.tile([P, T, FC, P], f16)

        for t in range(T):
            r0 = t * P
            dn_t = pk_sb[:, t, 1 : 1 + K]
            esc_t = pk_sb[:, t, 11 : 11 + K]

            # gather int8 embed rows: eg8[p, k, :] = E8[idx[r0+p, k], :]
            eg8 = eg_pool.tile([P, K, F], i8, tag="eg8")
            for k in range(K):
                nc.gpsimd.indirect_dma_start(
                    out=eg8[:, k, :],
                    out_offset=None,
                    in_=emb_full[:],
                    in_offset=bass.IndirectOffsetOnAxis(
                        ap=pk_i32[:, t, 21 + k : 22 + k], axis=0
                    ),
                )
            # int8 -> fp16 for the PE (scales are folded into the diag weights)
            eg = eg_pool.tile([P, K, F], f16, tag="eg")
            nc.vector.tensor_copy(out=eg[:], in_=eg8[:])

            # g_ik = s_ik * (Ê[i,k,:] . a2') ; f_i = X[i,:] . (W@a1)
            g_t = sm.tile([P, K], f32, tag="g")
            for k in range(K):
                ekf = scr.tile([P, F], f32, tag="ekf")
                nc.vector.tensor_copy(out=ekf[:], in_=eg8[:, k, :])
                dot(ekf[:], a2pb[:], g_t[:, k : k + 1])
            nc.vector.tensor_tensor(out=g_t[:], in0=g_t[:], in1=esc_t, op=OP.mult)

            # scores: s = leakyrelu(g + f) + dup_mask_neg
            s_t = sm.tile([P, K], f32, tag="s")
            nc.vector.tensor_scalar_add(out=s_t[:], in0=g_t[:], scalar1=pk_sb[:, t, 0:1])
            lr = sm.tile([P, K], f32, tag="lr")
            nc.vector.tensor_scalar_mul(out=lr[:], in0=s_t[:], scalar1=ALPHA)
            nc.vector.tensor_tensor(out=s_t[:], in0=s_t[:], in1=lr[:], op=OP.max)
            nc.vector.tensor_tensor(out=s_t[:], in0=s_t[:], in1=dn_t, op=OP.add)

            # masked softmax over k (exp and normalizer fused on ACT)
            mx = sm.tile([P, 1], f32, tag="mx")
            nc.vector.tensor_reduce(out=mx[:], in_=s_t[:], axis=AX.X, op=OP.max)
            nmx = sm.tile([P, 1], f32, tag="nmx")
            nc.vector.tensor_scalar_mul(out=nmx[:], in0=mx[:], scalar1=-1.0)
            p_t = sm.tile([P, K], f32, tag="p")
            z_t = sm.tile([P, 1], f32, tag="z")
            nc.scalar.activation(
                out=p_t[:], in_=s_t[:], func=ACT.Exp, bias=nmx[:], scale=1.0,
                accum_out=z_t[:],
            )
            zi = sm.tile([P, 1], f32, tag="zi")
            nc.vector.reciprocal(out=zi[:], in_=z_t[:])
            wts = sm.tile([P, K], f32, tag="wts")
            nc.vector.tensor_scalar_mul(out=wts[:], in0=p_t[:], scalar1=zi[:])
            # fold dequant scales into the aggregation weights:
            #   agg_i = sum_k (w_ik * s_ik) * Ê[idx_ik]
            nc.vector.tensor_tensor(out=wts[:], in0=wts[:], in1=esc_t, op=OP.mult)

            # diag(w) tiles (fp16 for the PE)
            dks = []
            for k in range(K):
                dk = dg.tile([P, P], f16, tag="dk")
                nc.vector.tensor_scalar_mul(out=dk[:], in0=ident[:], scalar1=wts[:, k : k + 1])
                dks.append(dk)

            # aggregation, transposed directly:
            #   aggT[m, n] = sum_k (eg[:, k, c*128+m]).T @ diag(w_k) = w_n * E[idx[n,k], c*128+m]
            for c in range(FC):
                at_ps = ps.tile([P, P], f32, tag="at_ps")
                for k in range(K):
                    nc.tensor.matmul(
                        out=at_ps[:],
                        lhsT=eg[:, k, c * P : (c + 1) * P],
                        rhs=dks[k][:],
                        start=(k == 0),
                        stop=(k == K - 1),
                    )
                nc.vector.tensor_copy(out=aggT[:, t, c, :], in_=at_ps[:])

            # out = agg @ W: out[r, j] = sum_c aggT[:, t, c, r] . W-chunk
            obf = scr.tile([P, F], f32, tag="obf")
            for nh in range(F // NH):
                o_ps = pso.tile([P, NH], f32, tag="o_ps")
                for c in range(FC):
                    nc.tensor.matmul(
                        out=o_ps[:],
                        lhsT=aggT[:, t, c, :],
                        rhs=w_sb[:, c, nh * NH : (nh + 1) * NH],
                        start=(c == 0),
                        stop=(c == FC - 1),
                    )
                nc.vector.tensor_copy(out=obf[:, nh * NH : (nh + 1) * NH], in_=o_ps[:])

            # int8 per-row quantization of the output (halves the slow D2H):
            #   osc_i = max|out_i|/127,  out8_i = out_i / osc_i
            omx = sm.tile([P, 1], f32, tag="omx")
            nc.vector.tensor_reduce(out=omx[:], in_=obf[:], axis=AX.X, op=OP.max)
            omn = sm.tile([P, 1], f32, tag="omn")
            nc.vector.tensor_reduce(out=omn[:], in_=obf[:], axis=AX.X, op=OP.min)
            nmn = sm.tile([P, 1], f32, tag="nmn")
            nc.vector.tensor_scalar_mul(out=nmn[:], in0=omn[:], scalar1=-1.0)
            rmax = sm.tile([P, 1], f32, tag="rmax")
            nc.vector.tensor_tensor(out=rmax[:], in0=omx[:], in1=nmn[:], op=OP.max)
            osc_t = sm.tile([P, 1], f32, tag="osc")
            nc.vector.tensor_scalar_mul(out=osc_t[:], in0=rmax[:], scalar1=1.0 / 127.0)
            nc.vector.tensor_scalar_max(out=osc_t[:], in0=osc_t[:], scalar1=1e-12)
            oinv = sm.tile([P, 1], f32, tag="oinv")
            nc.vector.reciprocal(out=oinv[:], in_=osc_t[:])
            o8 = scr.tile([P, F], i8, tag="o8")
            nc.vector.tensor_scalar_mul(out=o8[:], in0=obf[:], scalar1=oinv[:])
            nc.sync.dma_start(out8[r0 : r0 + P, :], o8[:])
            nc.sync.dma_start(osc[r0 : r0 + P, :], osc_t[:])

    nc.compile()
    return nc


_NC_CACHE = None


def _get_nc():
    global _NC_CACHE
    if _NC_CACHE is None:
        _NC_CACHE = build_kernel()
    return _NC_CACHE


def _host_prep(feature_matrix, embed_matrix, weight, a, neigh_idx, put=None):
    """Global (already concat-ordered) per-name arrays for the sharded runner.

    Every tensor is sharded on axis 0 across the 8 cores, so the full input
    arrays ARE the concatenation of per-core shards — no host copies beyond
    the fp16 casts. Only `av2` (replicated) needs tiling.

    If `put` is given, each big array is handed to it as soon as it is ready
    (async jax.device_put) so the wire transfer overlaps the remaining host
    prep.
    """
    if put is None:
        put = lambda x: x
    out = {}
    emb32 = np.asarray(embed_matrix, dtype=np.float32)
    # int8 per-row symmetric quantization of the embed table. Chunked so each
    # upload chunk starts its (async) wire transfer as soon as it is
    # quantized, and the tunnel runs several concurrent streams (~20% faster
    # than one large transfer). The per-row scales travel separately and are
    # folded into the attention weights on device, so only 1 byte/element
    # crosses the wire.
    esc = np.empty((M, 1), np.float32)
    import concurrent.futures as _cf

    def _quant_part(args):
        k, c, dst = args
        sl = slice(k * MSH + c * CHR, k * MSH + (c + 1) * CHR)
        ch = emb32[sl]
        sc = np.maximum(np.abs(ch).max(axis=1, keepdims=True), 1e-12) / 127.0
        dst[k * CHR : (k + 1) * CHR] = np.clip(np.round(ch / sc), -127, 127)
        esc[sl] = sc

    with _cf.ThreadPoolExecutor(8) as ex:
        for c in range(EMB_CH):
            dst = np.empty((NCORES * CHR, F), np.int8)
            list(ex.map(_quant_part, [(k, c, dst) for k in range(NCORES)]))
            out[f"emb_sh{c}"] = put(dst)
    w32 = np.asarray(weight, dtype=np.float32)
    out["w_sh"] = put(np.ascontiguousarray(w32.astype(np.float16)))

    feat32 = np.asarray(feature_matrix, dtype=np.float32)
    av = np.asarray(a, dtype=np.float32).reshape(2 * F)

    idx = np.asarray(neigh_idx)
    idx32 = idx.astype(np.int32)
    # duplicate-index mask (set semantics): only first occurrence is valid
    dup = np.zeros((N, K), dtype=bool)
    for k in range(1, K):
        dup[:, k] = (idx[:, :k] == idx[:, k : k + 1]).any(axis=1)

    # packed per-row metadata (one upload instead of four — each extra
    # sharded array costs ~10ms of tunnel overhead):
    pk = np.zeros((N, 36), np.float32)
    # f_i = X[i] . (W @ a1) — per-row logit offset (8KB instead of a 4MB
    # feature upload; the a2 half of the attention vector stays on device)
    pk[:, 0] = feat32 @ (w32 @ av[:F])
    pk[:, 1 : 1 + K] = np.where(dup, np.float32(NEGBIG), np.float32(0.0))
    pk[:, 11 : 11 + K] = esc[idx32, 0]  # per-(row,k) dequant scales
    pk[:, 21 : 21 + K] = idx32.view(np.float32)  # int32 bits, bitcast on device
    pk[:, 32:36] = np.tile((w32 @ av[F:]).astype(np.float32).reshape(NL, 4), (NCORES, 1))
    out["pk"] = put(pk)
    return out


_RUNNER_CACHE = None


def _make_runner(nc):
    """jit(shard_map(bass_exec)) caller, like bass2jax.run_bass_via_pjrt but:
    - zero output buffers are created ON DEVICE (no host->device zeros upload)
    - takes global arrays directly (no per-core split + re-concat on host)
    """
    import jax
    import jax.numpy as jnp
    from jax.experimental.shard_map import shard_map
    from jax.sharding import Mesh, PartitionSpec

    from concourse import bass2jax, mybir as _mybir
    from concourse.bass2jax import (
        _bass_exec_p,
        install_neuronx_cc_hook,
        partition_id_tensor,
    )

    install_neuronx_cc_hook()

    partition_name = nc.partition_id_tensor.name if nc.partition_id_tensor else None
    in_names, out_names, out_avals = [], [], []
    for alloc in nc.m.functions[0].allocations:
        if not isinstance(alloc, _mybir.MemoryLocationSet):
            continue
        name = alloc.memorylocations[0].name
        if alloc.kind == "ExternalInput":
            if name != partition_name:
                in_names.append(name)
        elif alloc.kind == "ExternalOutput":
            shape = tuple(alloc.tensor_shape)
            dtype = _mybir.dt.np(alloc.dtype)
            out_names.append(name)
            out_avals.append(jax.core.ShapedArray(shape, dtype))
    n_params = len(in_names)
    all_names = list(in_names) + list(out_names)
    if partition_name is not None:
        all_names.append(partition_name)

    def _body(*args):
        operands = list(args)
        if partition_name is not None:
            operands.append(partition_id_tensor())
        outs = _bass_exec_p.bind(
            *operands,
            out_avals=tuple(out_avals),
            in_names=tuple(all_names),
            out_names=tuple(out_names),
            lowering_input_output_aliases=(),
            sim_require_finite=True,
            sim_require_nnan=True,
            nc=nc,
        )
        return tuple(outs)

    devices = jax.devices()[:NCORES]
    mesh = Mesh(np.asarray(devices), ("core",))
    n_outs = len(out_names)
    sharded = jax.jit(
        shard_map(
            _body,
            mesh=mesh,
            in_specs=(PartitionSpec("core"),) * (n_params + n_outs),
            out_specs=(PartitionSpec("core"),) * n_outs,
            check_rep=False,
        )
    )

    # Persistent device-resident zero buffers for the NEFF output slots:
    # uploaded once here, reused every call (NOT donated, so never
    # invalidated). The kernel writes every element of every output, so
    # stale contents between calls are harmless.
    from jax.sharding import NamedSharding

    sh = NamedSharding(mesh, PartitionSpec("core"))
    zeros_dev = [
        jax.device_put(
            np.zeros((NCORES * a.shape[0], *a.shape[1:]), a.dtype), sh
        )
        for a in out_avals
    ]

    def call(name_map):
        args = [name_map[n] for n in in_names]
        outs = sharded(*args, *zeros_dev)
        return {n: outs[i] for i, n in enumerate(out_names)}

    return call, sh


def _get_runner():
    global _RUNNER_CACHE
    if _RUNNER_CACHE is None:
        _RUNNER_CACHE = _make_runner(_get_nc())
    return _RUNNER_CACHE


def run(inputs, trace=False, **kw):
    if trace:  # profiling path: go through the stock spmd runner
        nc = _get_nc()
        g = _host_prep(**inputs)
        in_maps = []
        for c in range(NCORES):
            m = {
                f"emb_sh{cc}": g[f"emb_sh{cc}"][c * CHR : (c + 1) * CHR]
                for cc in range(EMB_CH)
            }
            in_maps.append(
                {
                    **m,
                    "w_sh": g["w_sh"][c * WSH : (c + 1) * WSH],
                    "pk": g["pk"][c * NL : (c + 1) * NL],
                }
            )
        res = run_bass_kernel_spmd(
            nc, in_maps, core_ids=list(range(NCORES)), trace=trace, **kw
        )
        out = np.concatenate(
            [
                res.results[c]["out8"].astype(np.float32) * res.results[c]["osc"]
                for c in range(NCORES)
            ],
            axis=0,
        )
        return out, res
    global _DEVICE_POISONED
    if not _DEVICE_POISONED:
        try:
            return _run_device(inputs), None
        except Exception as e:
            # A device-unrecoverable error poisons this process's PJRT client
            # for good; all further device work goes through a fresh-process
            # worker (which CAN recover, since it gets a fresh client).
            sys.stderr.write(
                f"kernel: in-process device call failed ({type(e).__name__}); "
                f"switching to subprocess worker\n"
            )
            _DEVICE_POISONED = True
    prepped = _host_prep(**inputs)
    for attempt in range(2):
        try:
            return _worker_call(prepped), None
        except Exception as e:
            sys.stderr.write(
                f"kernel: worker attempt {attempt} failed ({type(e).__name__})\n"
            )
            _kill_worker()
    sys.stderr.write("kernel: all device paths failed; host fallback\n")
    return _numpy_fallback(**inputs), None


def _run_device(inputs):
    import concurrent.futures as _cf

    import jax

    call, sh = _get_runner()
    put = lambda arr: jax.device_put(arr, sh)
    outs = call(_host_prep(**inputs, put=put))
    return _fetch_out(outs)


def _fetch_out(outs):
    import concurrent.futures as _cf

    # start all shard->host copies, then fetch both outputs concurrently
    # (each D2H fetch has ~0.1s fixed latency)
    for s in outs["out8"].addressable_shards:
        try:
            s.data.copy_to_host_async()
        except Exception:
            break
    with _cf.ThreadPoolExecutor(2) as ex:
        f8 = ex.submit(np.asarray, outs["out8"])
        fs = ex.submit(np.asarray, outs["osc"])
        o8, sc = f8.result(), fs.result()
    return o8.astype(np.float32) * sc


def _exec_prepped(prepped):
    """Device execution for an already-prepped numpy dict (worker path)."""
    import jax

    call, sh = _get_runner()
    staged = {k: jax.device_put(v, sh) for k, v in prepped.items()}
    return _fetch_out(call(staged))


_DEVICE_POISONED = False
_WORKER = None


def _spawn_worker():
    import subprocess

    here = os.path.dirname(os.path.abspath(__file__))
    boot = (
        "import sys; "
        f"sys.path.insert(0, {here!r}); "
        "sys.path.insert(0, '/opt/trn_rl_repo'); "
        "import kernel; kernel._worker_main()"
    )
    return subprocess.Popen(
        [sys.executable, "-u", "-c", boot],
        stdin=subprocess.PIPE,
        stdout=subprocess.PIPE,
        stderr=sys.stderr,
        text=True,
    )


def _kill_worker():
    global _WORKER
    if _WORKER is not None:
        try:
            _WORKER.kill()
        except Exception:
            pass
        _WORKER = None


def _worker_call(prepped, timeout=420.0):
    global _WORKER
    import tempfile
    import threading

    if _WORKER is None or _WORKER.poll() is not None:
        _kill_worker()
        _WORKER = _spawn_worker()
        import atexit

        atexit.register(_kill_worker)
    proc = _WORKER
    tmp = tempfile.mkdtemp()
    in_path = os.path.join(tmp, "in.npz")
    out_path = os.path.join(tmp, "out.npy")
    np.savez(in_path, **prepped)
    proc.stdin.write(f"CALL {in_path} {out_path}\n")
    proc.stdin.flush()
    timer = threading.Timer(timeout, proc.kill)
    timer.start()
    try:
        while True:
            line = proc.stdout.readline()
            if not line:
                raise RuntimeError("worker died")
            if line.startswith("DONE"):
                return np.load(out_path)
            if line.startswith("FAIL"):
                raise RuntimeError(f"worker: {line.strip()}")
            # anything else is stray log noise on stdout; skip it
    finally:
        timer.cancel()


def _worker_main():
    """Entry point for the device-worker subprocess (fresh PJRT client)."""
    for line in sys.stdin:
        parts = line.split()
        if not parts:
            continue
        if parts[0] == "QUIT":
            break
        if parts[0] != "CALL" or len(parts) != 3:
            continue
        in_path, out_path = parts[1], parts[2]
        try:
            data = np.load(in_path)
            prepped = {k: data[k] for k in data.files}
            out = _exec_prepped(prepped)
            np.save(out_path, out)
            sys.stdout.write("DONE\n")
        except Exception as e:
            sys.stdout.write(f"FAIL {type(e).__name__}: {str(e)[:120]}".replace("\n", " ") + "\n")
        sys.stdout.flush()


def _numpy_fallback(feature_matrix, embed_matrix, weight, a, neigh_idx):
    """Exceptional-path-only host implementation (exact, sparse formulation)."""
    X = np.asarray(feature_matrix, dtype=np.float32)
    E = np.asarray(embed_matrix, dtype=np.float32)
    W = np.asarray(weight, dtype=np.float32)
    av = np.asarray(a, dtype=np.float32).reshape(2 * F)
    idx = np.asarray(neigh_idx)
    fv = X @ (W @ av[:F])
    a2p = W @ av[F:]
    eg = E[idx]  # [N, K, F]
    g = eg @ a2p
    s = fv[:, None] + g
    s = np.where(s > 0, s, np.float32(ALPHA) * s)
    dup = np.zeros((N, K), dtype=bool)
    for k in range(1, K):
        dup[:, k] = (idx[:, :k] == idx[:, k : k + 1]).any(axis=1)
    s = np.where(dup, np.float32(NEGBIG), s)
    s = s - s.max(axis=1, keepdims=True)
    p = np.exp(s)
    p /= p.sum(axis=1, keepdims=True)
    agg = np.einsum("nk,nkf->nf", p, eg)
    return (agg @ W).astype(np.float32)


def kernel(**inputs) -> np.ndarray:
    out, _ = run(inputs, trace=False)
    return out
